# revision 8
# baseline (speedup 1.0000x reference)
"""Causal attention (softmax over the *query* axis) for TRN2, 8 NeuronCores.

Problem: x:[8,2048,1024] f32, Wq/Wk/Wv:[1024,1024] f32.
  q/k/v = x @ W;  scores = q @ k^T  (causal: j>i masked)
  weights = softmax(scores/32, axis=1)   <-- normalized over the QUERY axis i
  out = weights @ v

Because the softmax normalizes per *column* j of the score matrix, the
computation factorizes exactly (no online softmax needed):
  E[i,j]  = exp(s[i,j]/32) * (j<=i)
  den[j]  = sum_i E[i,j]
  out     = E @ (v / den[:,None])

Sharding: pure data-parallel, one batch per core (8 batches, 8 cores).

Per-core kernel (all matmuls in bf16 with fp32 PSUM accumulation):
  A) x -> x^T   (PE transposes, stored bf16, layout [d, s])
  B) q^T = Wq^T x^T, k^T = Wk^T x^T   stored [e, s] bf16
  C) S^T tiles [128(j) x 512(i)] = (k^T block)^T @ q^T; additive causal mask
     on diagonal chunks; ScalarE exp((s+mask)/32) straight out of PSUM with
     fused free-axis accum -> per-column denominators; E stored bf16.
     Chunks fully left of the diagonal are skipped.
  D) v = (x^T)^T Wv fused with *1/den (per-partition scalar) -> v' bf16
  E) out[i-block] = sum_{J<=I} E_J^T @ v'_J  (PSUM accum), f32 out.
"""

import sys

if "/opt/trn_rl_repo" not in sys.path:
    sys.path.insert(0, "/opt/trn_rl_repo")

import numpy as np

import concourse.bass as bass
import concourse.tile as tile
from concourse import bacc, mybir
from concourse.bass_utils import run_bass_kernel_spmd
from concourse.masks import make_identity

P = 128
S = 2048          # sequence length
D = 1024          # d_in
E = 1024          # d_out
NS = S // P       # 16 s-tiles
ND = D // P       # 8 d-blocks
NE = E // P       # 8 e-blocks
CH = 512          # matmul free-dim chunk (= one fp32 PSUM bank)
NCH_S = S // CH   # 4
NCH_E = E // CH   # 2
SCALE = 1.0 / 32.0          # 1/sqrt(d_out)
MASK_VAL = -30000.0         # additive; exp((s+MASK_VAL)/32) == 0 in f32

F32 = mybir.dt.float32
BF16 = mybir.dt.bfloat16

N_CORES = 8
B = 8

# set by test harness for profiling; harness-default is plain run
_TRACE = False
_LAST_RESULT = None


def _copy(nc, idx, out, in_):
    """Alternate PSUM->SBUF copies between ScalarE and VectorE."""
    if idx % 2 == 0:
        nc.scalar.copy(out, in_)
    else:
        nc.vector.tensor_copy(out, in_)


def build_program() -> bass.Bass:
    nc = bacc.Bacc("TRN2", target_bir_lowering=False, debug=False)

    x_ext = nc.declare_dram_parameter("x", [S, D], F32, isOutput=False)
    wq_ext = nc.declare_dram_parameter("Wq", [D, E], F32, isOutput=False)
    wk_ext = nc.declare_dram_parameter("Wk", [D, E], F32, isOutput=False)
    wv_ext = nc.declare_dram_parameter("Wv", [D, E], F32, isOutput=False)
    out_ext = nc.declare_dram_parameter("out", [S, E], F32, isOutput=True)

    with tile.TileContext(nc) as tc:
        # ---- long-lived pools (left stack) ---------------------------------
        const_pool = tc.alloc_tile_pool(name="const", bufs=1, side="left")
        xT_pool = tc.alloc_tile_pool(name="xT", bufs=1, side="left")
        psum_pool = tc.alloc_tile_pool(name="psum", bufs=6, space="PSUM")

        # ---- constants -----------------------------------------------------
        ident = const_pool.tile([P, P], F32, tag="ident", name="ident")
        make_identity(nc, ident)

        # additive causal masks for the 4 diagonal-chunk phases, one tile:
        # masks[p, r*CH + f] = 0 if f >= 128*r + p else MASK_VAL
        masks = const_pool.tile([P, 4 * CH], F32, tag="masks", name="masks")
        for r in range(4):
            m = masks[:, r * CH : (r + 1) * CH]
            nc.gpsimd.memset(m, 0.0)
            nc.gpsimd.affine_select(
                out=m,
                in_=m,
                compare_op=mybir.AluOpType.is_ge,
                fill=MASK_VAL,
                base=-(P * r),
                pattern=[[1, CH]],
                channel_multiplier=-1,
            )

        # denominators: partial sums per (j_tile, chunk) in cols [0,64),
        # per-j-tile reciprocals in cols [64, 80), scratch col 80.
        denb = const_pool.tile(
            [P, NS * NCH_S + NS + 1], F32, tag="denb", name="denb"
        )
        dpart = denb[:, : NS * NCH_S]
        rden = denb[:, NS * NCH_S : NS * NCH_S + NS]
        dtmp = denb[:, NS * NCH_S + NS :]

        # x^T, bf16; block (t, d) at cols (t*ND + d)*P.
        xT = xT_pool.tile([P, NS * ND * P], BF16, tag="xT", name="xT")
        # view for phase-B rhs slicing: [p, t, d, c]
        xT_v = xT.rearrange("p (t d c) -> p t d c", t=NS, d=ND, c=P)

        # ---- phase A: load x, transpose to x^T -----------------------------
        xstage_pool = tc.alloc_tile_pool(name="xstage", bufs=4, side="right")
        tpsum_pool = tc.alloc_tile_pool(name="tpsum", bufs=2, space="PSUM")
        ci = 0
        for t in range(NS):
            xs = xstage_pool.tile([P, D], F32, tag="xs", name="xs")
            nc.sync.dma_start(out=xs, in_=x_ext[t * P : (t + 1) * P, :])
            for g in range(2):  # two groups of 4 d-blocks
                tp = tpsum_pool.tile([P, 4 * P], F32, tag="tp", name="tp")
                for q in range(4):
                    d = 4 * g + q
                    nc.tensor.transpose(
                        tp[:, q * P : (q + 1) * P],
                        xs[:, d * P : (d + 1) * P],
                        ident,
                    )
                c0 = (t * ND + 4 * g) * P
                _copy(nc, ci, xT[:, c0 : c0 + 4 * P], tp)
                ci += 1
        tpsum_pool.release()
        xstage_pool.release()

        # ---- phase B: weight loads + q^T / k^T projections -----------------
        wv_pool = tc.alloc_tile_pool(name="wv", bufs=1, side="left")
        qkT_pool = tc.alloc_tile_pool(name="qkT", bufs=1, side="left")
        wstage_pool = tc.alloc_tile_pool(name="wstage", bufs=3, side="right")
        wqk_pool = tc.alloc_tile_pool(name="wqk", bufs=1, side="right")

        wv_bf = wv_pool.tile([P, ND * E], BF16, tag="wv", name="wv")
        w_bf = {
            "q": wqk_pool.tile([P, ND * E], BF16, tag="wqb", name="wqb"),
            "k": wqk_pool.tile([P, ND * E], BF16, tag="wkb", name="wkb"),
        }
        ci = 0
        for ext, dst in ((wq_ext, w_bf["q"]), (wk_ext, w_bf["k"]), (wv_ext, wv_bf)):
            for d in range(ND):
                ws = wstage_pool.tile([P, E], F32, tag="ws", name="ws")
                nc.sync.dma_start(out=ws, in_=ext[d * P : (d + 1) * P, :])
                _copy(nc, ci, dst[:, d * E : (d + 1) * E], ws)
                ci += 1

        qkT = {
            "q": qkT_pool.tile([P, NE * S], BF16, tag="qT", name="qT"),
            "k": qkT_pool.tile([P, NE * S], BF16, tag="kT", name="kT"),
        }
        ci = 0
        for name in ("q", "k"):
            wt = w_bf[name]
            dst = qkT[name]
            for e in range(NE):
                for sc in range(NCH_S):
                    ps = psum_pool.tile([P, CH], F32, tag="ps", name="ps")
                    for d in range(ND):
                        nc.tensor.matmul(
                            ps,
                            lhsT=wt[:, d * E + e * P : d * E + (e + 1) * P],
                            rhs=xT_v[:, 4 * sc : 4 * sc + 4, d, :],
                            start=(d == 0),
                            stop=(d == ND - 1),
                        )
                    _copy(
                        nc,
                        ci,
                        dst[:, e * S + sc * CH : e * S + (sc + 1) * CH],
                        ps,
                    )
                    ci += 1
        wqk_pool.release()
        wstage_pool.release()

        # ---- phase C: scores^T, exp, denominators --------------------------
        E_pool = tc.alloc_tile_pool(name="Ee", bufs=1, side="right")
        # E^T, bf16; block (J, i) at cols J*S + i
        Ee = E_pool.tile([P, NS * S], BF16, tag="Ee", name="Ee")
        kT = qkT["k"]
        qT = qkT["q"]
        for J in range(NS):
            c_lo = J // NCH_S  # first chunk touching the diagonal
            for c in range(c_lo, NCH_S):
                ps = psum_pool.tile([P, CH], F32, tag="ps", name="ps")
                for e in range(NE):
                    nc.tensor.matmul(
                        ps,
                        lhsT=kT[:, e * S + J * P : e * S + (J + 1) * P],
                        rhs=qT[:, e * S + c * CH : e * S + (c + 1) * CH],
                        start=(e == 0),
                        stop=(e == NE - 1),
                    )
                if c == c_lo:
                    r = J % 4
                    nc.vector.tensor_add(ps, ps, masks[:, r * CH : (r + 1) * CH])
                nc.scalar.activation(
                    out=Ee[:, J * S + c * CH : J * S + (c + 1) * CH],
                    in_=ps,
                    func=mybir.ActivationFunctionType.Exp,
                    scale=SCALE,
                    accum_out=dpart[:, J * NCH_S + c : J * NCH_S + c + 1],
                )
            # denominator -> reciprocal for this j-tile
            nwid = NCH_S - c_lo
            src = dpart[:, J * NCH_S + c_lo : J * NCH_S + NCH_S]
            if nwid == 1:
                nc.vector.reciprocal(rden[:, J : J + 1], src)
            else:
                nc.vector.reduce_sum(dtmp, src, axis=mybir.AxisListType.X)
                nc.vector.reciprocal(rden[:, J : J + 1], dtmp)
        qkT_pool.release()

        # ---- phase D: v projection fused with 1/den ------------------------
        vp_pool = tc.alloc_tile_pool(name="vp", bufs=1, side="right")
        vp = vp_pool.tile([P, NS * E], BF16, tag="vp", name="vp")
        for J in range(NS):
            for ec in range(NCH_E):
                ps = psum_pool.tile([P, CH], F32, tag="ps", name="ps")
                for d in range(ND):
                    nc.tensor.matmul(
                        ps,
                        lhsT=xT[:, (J * ND + d) * P : (J * ND + d + 1) * P],
                        rhs=wv_bf[:, d * E + ec * CH : d * E + (ec + 1) * CH],
                        start=(d == 0),
                        stop=(d == ND - 1),
                    )
                nc.vector.tensor_scalar_mul(
                    vp[:, J * E + ec * CH : J * E + (ec + 1) * CH],
                    ps,
                    rden[:, J : J + 1],
                )

        # ---- phase E: out = E @ v' -----------------------------------------
        ostage_pool = tc.alloc_tile_pool(name="ostage", bufs=4, side="right")
        ci = 0
        for I in range(NS):
            for ec in range(NCH_E):
                ps = psum_pool.tile([P, CH], F32, tag="ps", name="ps")
                for J in range(I + 1):
                    nc.tensor.matmul(
                        ps,
                        lhsT=Ee[:, J * S + I * P : J * S + (I + 1) * P],
                        rhs=vp[:, J * E + ec * CH : J * E + (ec + 1) * CH],
                        start=(J == 0),
                        stop=(J == I),
                    )
                os_ = ostage_pool.tile([P, CH], F32, tag="os", name="os")
                _copy(nc, ci, os_, ps)
                ci += 1
                nc.sync.dma_start(
                    out=out_ext[I * P : (I + 1) * P, ec * CH : (ec + 1) * CH],
                    in_=os_,
                )
        ostage_pool.release()
        vp_pool.release()
        E_pool.release()
        wv_pool.release()
        xT_pool.release()
        psum_pool.release()
        const_pool.release()
    nc.compile()
    return nc


_PROGRAM = None


def _get_program():
    global _PROGRAM
    if _PROGRAM is None:
        _PROGRAM = build_program()
    return _PROGRAM


def kernel(**inputs: np.ndarray) -> np.ndarray:
    global _LAST_RESULT
    x = np.ascontiguousarray(np.asarray(inputs["x"], dtype=np.float32))
    wq = np.ascontiguousarray(np.asarray(inputs["Wq"], dtype=np.float32))
    wk = np.ascontiguousarray(np.asarray(inputs["Wk"], dtype=np.float32))
    wv = np.ascontiguousarray(np.asarray(inputs["Wv"], dtype=np.float32))
    assert x.shape == (B, S, D)

    nc = _get_program()
    in_maps = [
        {"x": x[b], "Wq": wq, "Wk": wk, "Wv": wv} for b in range(N_CORES)
    ]
    res = run_bass_kernel_spmd(nc, in_maps, list(range(N_CORES)), trace=_TRACE)
    _LAST_RESULT = res
    out = np.stack([res.results[b]["out"] for b in range(B)], axis=0)
    return out.astype(np.float32)


if __name__ == "__main__":
    rng = np.random.default_rng(0)
    ins = {
        "x": rng.standard_normal((B, S, D), dtype=np.float32),
        "Wq": rng.standard_normal((D, E), dtype=np.float32) * 0.02,
        "Wk": rng.standard_normal((D, E), dtype=np.float32) * 0.02,
        "Wv": rng.standard_normal((D, E), dtype=np.float32) * 0.02,
    }
    out = kernel(**ins)
    print("out", out.shape, out.dtype)
